# revision 14
# baseline (speedup 1.0000x reference)
"""Trainium2 Bass kernel for ChannelFeatures (channel-attention style module).

Computes, per batch element b:
    x_max[b] = max over (H,W) of features[b]          # (C,)
    x_avg[b] = mean over (H,W) of features[b]         # (C,)
    7 residual blocks (shared weights on both branches):
        x = prelu(W1[k] @ x + b1[k], a1[k]) + x
    scores[b] = sigmoid(x_max[b] + x_avg[b])          # (C,)
    out[b] = features[b] * scores[b]                  # broadcast over (H,W)

Sharding: pure data parallel over batch — 16 batch elements across 8 cores,
2 per core, weights replicated. No cross-core communication.

Device strategy per core (2 batch elements, each (65536, 64) fp32):
  Pass 1: stream (128, KF, 64) tiles; DVE reduce (max / sum) over the KF axis
          into per-tile partials; combine partials; cross-partition max via
          PE transpose + DVE reduce; cross-partition mean via ones-matmul
          (rhs preloaded with 1/HW so the mean scale is folded in).
  Recurrence: channels on partitions, both branches x both batches as a
          (64, 4) tile; 7x (PE matmul + bias via ACT + prelu via DVE
          tensor_scalar min/max + residual add).
  Pass 2: re-stream tiles, multiply by the per-batch broadcast score row,
          DMA back out.
"""

import numpy as np
from contextlib import ExitStack

import concourse.bass as bass
import concourse.tile as tile
from concourse import masks, mybir
from concourse.bass_utils import run_bass_kernel_spmd

# Problem shapes (hardcoded per contract)
B, H, W, C = 16, 256, 256, 64
CONV_NUM = 7
NCORES = 8
BPC = B // NCORES          # batch elements per core
HW = H * W                 # 65536 spatial positions
P = 128                    # SBUF partitions
KF = 32                    # spatial rows per partition per tile
TILE_ROWS = P * KF         # 4096 spatial rows per tile
T = HW // TILE_ROWS        # 16 tiles per batch element
F32 = mybir.dt.float32

# test.py hooks: set PROFILE=True before calling kernel() to capture an NTFF
# trace; LAST_EXEC_NS then holds the max per-core HW execution time.
PROFILE = False
LAST_EXEC_NS = None


def _split_dma_waits(nc: bass.Bass) -> None:
    """The pinned walrus build rejects DMA instructions carrying more than one
    sync-wait ("Too many sync wait commands"). Tile's sem assignment is not
    transitively minimal, so slot-reuse instructions can get two waits
    (consumer release + WAW with the previous writer). Hoist all but the last
    wait onto wait-only EventSemaphore instructions on the same engine right
    before the instruction."""
    n = 0
    # num=200: outside every id Tile allocated (its end-of-kernel range-clear
    # covers the allocated block), so no collision with released Tile sems.
    dummy = nc.alloc_semaphore(name="wsplit_dummy", num=200)
    for fn in nc.m.functions:
        for blk in fn.blocks:
            new_insts = []
            for inst in blk.instructions:
                si = getattr(inst, "sync_info", None)
                if si is not None and len(si.on_wait) > 1:
                    for w in si.on_wait[:-1]:
                        ev = mybir.InstEventSemaphore(
                            name=f"WSPLIT-{n}", ins=[], outs=[]
                        )
                        n += 1
                        ev.engine = inst.engine
                        # Tick a dedicated dummy sem nobody waits on, so the
                        # simulator/race tooling (which require every
                        # instruction to carry an update) accept the carrier.
                        upd = mybir.SyncUpdate(
                            sync_type="semaphore",
                            id=dummy.num,
                            ant_name=dummy.name,
                            update_mode="sem-add-imm",
                            update_value=1,
                        )
                        ev.sync_info = mybir.SyncInfo(on_wait=[w], on_update=[upd])
                        new_insts.append(ev)
                    si.on_wait = [si.on_wait[-1]]
                new_insts.append(inst)
            blk.instructions = new_insts


def _build_nc() -> bass.Bass:
    nc = bass.Bass()
    feat = nc.declare_dram_parameter("features", [BPC, HW, C], F32, isOutput=False)
    wT = nc.declare_dram_parameter("wT", [C, CONV_NUM, C], F32, isOutput=False)
    bT = nc.declare_dram_parameter("bT", [C, CONV_NUM], F32, isOutput=False)
    aT = nc.declare_dram_parameter("aT", [C, CONV_NUM], F32, isOutput=False)
    out = nc.declare_dram_parameter("out", [BPC, HW, C], F32, isOutput=True)
    scores_dram = nc.dram_tensor("scores_scratch", [BPC, C], F32)

    feat_t = feat[:].rearrange("b (t p k) c -> b t p k c", p=P, k=KF)
    out_t = out[:].rearrange("b (t p k) c -> b t p k c", p=P, k=KF)

    with ExitStack() as ctx:
        tc = ctx.enter_context(tile.TileContext(nc))
        singles = ctx.enter_context(tc.tile_pool(name="singles", bufs=1))
        stream = ctx.enter_context(tc.tile_pool(name="stream", bufs=4))
        partials = ctx.enter_context(tc.tile_pool(name="partials", bufs=2))
        psum = ctx.enter_context(tc.tile_pool(name="psum", bufs=2, space="PSUM"))
        small = ctx.enter_context(tc.tile_pool(name="small", bufs=2))

        # Constants
        w_sb = singles.tile([C, CONV_NUM, C], F32)   # [c_in, k, c_out]
        nc.sync.dma_start(out=w_sb[:], in_=wT[:])
        b_sb = singles.tile([C, CONV_NUM], F32)      # [c, k]
        nc.sync.dma_start(out=b_sb[:], in_=bT[:])
        a_sb = singles.tile([C, CONV_NUM], F32)      # [c, k] (a1[k] per row)
        nc.sync.dma_start(out=a_sb[:], in_=aT[:])
        identity = singles.tile([P, P], F32)
        masks.make_identity(nc, identity[:])
        inv_hw = singles.tile([P, 1], F32)
        nc.vector.memset(inv_hw[:], 1.0 / HW)

        # [channel, branch(0=max,1=avg), batch]
        xvec = singles.tile([C, 2, BPC], F32)

        # ---- Pass 1: reductions ----
        for b in range(BPC):
            maxp = partials.tile([P, T, C], F32)
            sump = partials.tile([P, T, C], F32)
            for t in range(T):
                tl = stream.tile([P, KF, C], F32)
                nc.gpsimd.dma_start(out=tl[:], in_=feat_t[b, t])
                tv = tl[:].transpose([0, 2, 1])  # (P, C, KF) strided view
                nc.vector.reduce_max(out=maxp[:, t, :], in_=tv, axis=mybir.AxisListType.X)
                nc.vector.reduce_sum(out=sump[:, t, :], in_=tv, axis=mybir.AxisListType.X)
            maxr = small.tile([P, C], F32)
            nc.vector.reduce_max(
                out=maxr[:], in_=maxp[:].transpose([0, 2, 1]), axis=mybir.AxisListType.X
            )
            sumr = small.tile([P, C], F32)
            nc.vector.reduce_sum(
                out=sumr[:], in_=sump[:].transpose([0, 2, 1]), axis=mybir.AxisListType.X
            )
            # cross-partition max: PE transpose (P,C)->(C,P), DVE reduce
            mt = psum.tile([C, P], F32)
            nc.tensor.transpose(mt[:], maxr[:], identity[:])
            nc.vector.reduce_max(
                out=xvec[:, 0, b : b + 1], in_=mt[:], axis=mybir.AxisListType.X
            )
            # cross-partition mean: sumr.T @ (1/HW ones)  -> (C, 1)
            st = psum.tile([C, 1], F32)
            nc.tensor.matmul(st[:], sumr[:], inv_hw[:], start=True, stop=True)
            nc.scalar.copy(out=xvec[:, 1, b : b + 1], in_=st[:])

        # ---- Recurrence: 7 residual PReLU blocks on (C, 2*BPC) ----
        xf = xvec[:].rearrange("c r b -> c (r b)")  # (C, 4); cols: branch-major
        for k in range(CONV_NUM):
            y = psum.tile([C, 2 * BPC], F32)
            nc.tensor.matmul(y[:], w_sb[:, k, :], xf, start=True, stop=True)
            z = small.tile([C, 2 * BPC], F32)
            nc.scalar.activation(
                out=z[:],
                in_=y[:],
                func=mybir.ActivationFunctionType.Identity,
                bias=b_sb[:, k : k + 1],
                scale=1.0,
            )
            pos = small.tile([C, 2 * BPC], F32)
            nc.vector.tensor_scalar_max(pos[:], z[:], 0.0)
            neg = small.tile([C, 2 * BPC], F32)
            nc.vector.tensor_scalar(
                neg[:],
                z[:],
                0.0,
                a_sb[:, k : k + 1],
                mybir.AluOpType.min,
                mybir.AluOpType.mult,
            )
            pn = small.tile([C, 2 * BPC], F32)
            nc.vector.tensor_add(pn[:], pos[:], neg[:])
            xn = small.tile([C, 2 * BPC], F32)
            nc.vector.tensor_add(xn[:], pn[:], xf)
            xf = xn[:]

        # scores = sigmoid(x_max + x_avg): (C, BPC)
        ssum = small.tile([C, BPC], F32)
        nc.vector.tensor_add(ssum[:], xf[:, 0:BPC], xf[:, BPC : 2 * BPC])
        scores = small.tile([C, BPC], F32)
        nc.scalar.activation(
            out=scores[:], in_=ssum[:], func=mybir.ActivationFunctionType.Sigmoid
        )
        # transpose (C, BPC) -> (BPC, C) and broadcast each row to 128 partitions
        sc_t = psum.tile([BPC, C], F32)
        nc.tensor.transpose(sc_t[:], scores[:], identity[:C, :C])
        sc_sb = small.tile([BPC, C], F32)
        nc.vector.tensor_copy(sc_sb[:], sc_t[:])
        nc.gpsimd.dma_start(out=scores_dram[:], in_=sc_sb[:])
        bc = []
        for b in range(BPC):
            bcb = singles.tile([P, C], F32, tag=f"bc{b}")
            nc.gpsimd.dma_start(
                out=bcb[:], in_=scores_dram[b : b + 1, :].to_broadcast([P, C])
            )
            bc.append(bcb)

        # ---- Pass 2: scale and write out ----
        for b in range(BPC):
            bcv = bc[b][:].unsqueeze(1).to_broadcast([P, KF, C])
            for t in range(T):
                tl = stream.tile([P, KF, C], F32)
                nc.gpsimd.dma_start(out=tl[:], in_=feat_t[b, t])
                nc.vector.tensor_mul(tl[:], tl[:], bcv)
                nc.gpsimd.dma_start(out=out_t[b, t], in_=tl[:])

    _split_dma_waits(nc)
    return nc


def _prep_inputs(features, W1, b1, a1):
    feats = np.ascontiguousarray(features, dtype=np.float32).reshape(B, HW, C)
    # lhsT layout: wT[c_in, k, c_out] = W1[k, c_out, c_in]
    wT = np.ascontiguousarray(np.transpose(np.asarray(W1, np.float32), (2, 0, 1)))
    bT = np.ascontiguousarray(np.asarray(b1, np.float32).T)            # (C, 7)
    aT = np.ascontiguousarray(
        np.broadcast_to(np.asarray(a1, np.float32), (C, CONV_NUM))
    )
    return feats, wT, bT, aT


def kernel(features, W1, b1, a1):
    global LAST_EXEC_NS
    feats, wT, bT, aT = _prep_inputs(features, W1, b1, a1)
    nc = _build_nc()
    in_maps = [
        {
            "features": feats[i * BPC : (i + 1) * BPC],
            "wT": wT,
            "bT": bT,
            "aT": aT,
        }
        for i in range(NCORES)
    ]
    res = run_bass_kernel_spmd(nc, in_maps, list(range(NCORES)), trace=PROFILE)
    LAST_EXEC_NS = res.exec_time_ns
    out = np.concatenate(
        [res.results[i]["out"].reshape(BPC, H, W, C) for i in range(NCORES)], axis=0
    )
    return out
